# revision 4
# baseline (speedup 1.0000x reference)
"""DistanceLoss v2 — bf16-packed DVE pipeline, ACT offload, fused transforms.

Changes vs baseline kernel.py:
  - min-plus runs on bf16 operands (STT if 2x-packed, else TS-biased + TT-min
    chain, both exact: all in-band values are small integers, exact in bf16)
  - Square folded into the PSUM->SBUF copies on ACT (g transposed raw, squared
    during the copy); separate ACT Square ops and f32 g2T tiles removed
  - softmax pipeline in bf16 (exp out, class sums, q, eq) for DVE 2x packing
  - S2 mask from d2p (present <=> d2 == 0) removing the t_nat dependency
  - maxd2 via one strided X-reduce -> [P, 10] per sample
  - scan dtype switchable f32/bf16 (SCAN_F32)
"""

import numpy as np

B, C, H, W = 16, 5, 256, 256
NCORES = 8
BPC = B // NCORES
R = 6
BIG = 512.0
P = 128

# variant flags (set from measurements: bf16 STT is 1x, TS is 4x, TT is 2x,
# f32 and bf16 scans are the same speed)
SCAN_F32 = False

_CACHE = {}


def _build_nc(legalize=True, race_detect=True):
    import concourse.bass as bass
    import concourse.mybir as mybir
    import concourse.tile as tile

    f32 = mybir.dt.float32
    i32 = mybir.dt.int32
    bf16 = mybir.dt.bfloat16
    Alu = mybir.AluOpType
    Act = mybir.ActivationFunctionType

    sdt = f32 if SCAN_F32 else bf16

    nc = bass.Bass(detect_race_conditions=race_detect)
    pred_d = nc.dram_tensor("predictions", [BPC, C, H, W], f32, kind="ExternalInput")
    tgt_d = nc.dram_tensor("targets", [BPC, H, W], i32, kind="ExternalInput")
    # stats columns: [0:10] S1 (b*5+c), [10:20] S2, [20:40] maxd2 (b*10 + a*5+c)
    out_d = nc.dram_tensor("out_stats", [P, 48], f32, kind="ExternalOutput")

    SCW = H + 2
    CW = W + 8
    TL = 2 * C * CW

    with tile.TileContext(nc) as tc:
        with (
            tc.tile_pool(name="const", bufs=1) as cpool,
            tc.tile_pool(name="work", bufs=2) as pool,
            tc.tile_pool(name="dmabuf", bufs=2) as dpool,
            tc.tile_pool(name="psum", bufs=4, space="PSUM") as psum,
        ):
            # 128x128 identity built via DVE, ACT-copied (PE stationary input)
            onesf = cpool.tile([P, P], f32)
            nc.vector.memset(onesf[:], 1.0)
            rowv = cpool.tile([P, P], f32)
            nc.vector.tensor_tensor_scan(
                rowv[:], onesf[:], onesf[:], 0.0, Alu.add, Alu.mult)
            colv = cpool.tile([P, 1], f32)
            colm = cpool.tile([P, 32], f32)
            for a in range(4):
                nc.vector.transpose(
                    colm[a * 32:(a + 1) * 32, :],
                    rowv[a * 32:(a + 1) * 32, a * 32:(a + 1) * 32])
            nc.vector.tensor_copy(colv[:], colm[:, :1])
            identD = cpool.tile([P, P], f32)
            nc.vector.tensor_scalar(
                identD[:], rowv[:], colv[:], None, Alu.is_equal)
            ident = cpool.tile([P, P], f32)
            nc.scalar.copy(ident[:], identD[:])
            identB = cpool.tile([P, P], bf16)
            nc.scalar.copy(identB[:], identD[:])

            # warm the ACT function tables while DMAs are in flight
            warm = cpool.tile([P, 4], f32)
            nc.scalar.activation(warm[:, 0:1], onesf[:, :1], Act.Exp)
            nc.scalar.activation(warm[:, 1:2], onesf[:, :1], Act.Ln)
            nc.scalar.activation(warm[:, 2:3], onesf[:, :1], Act.Sqrt)
            nc.scalar.activation(warm[:, 3:4], onesf[:, :1], Act.Square)

            ones_s = cpool.tile([P, 2 * C * SCW], sdt)

            stats = cpool.tile([P, 48], f32)
            nc.vector.memset(stats[:], 0.0)

            # both target DMAs go out before the (much larger) pred DMAs so
            # sample 1's cast never stalls behind sample 0's pred transfers
            t_i32s = []
            for b in range(BPC):
                t_i32 = dpool.tile([P, 2, W], i32, tag=f"tgt{b}")
                nc.sync.dma_start(
                    t_i32[:], tgt_d[b].rearrange("(n p) w -> p n w", p=P))
                t_i32s.append(t_i32)

            for b in range(BPC):
                # ---- targets: cast, transpose to [w, wb, h] ----
                t_i32 = t_i32s[b]
                t_nat = pool.tile([P, 2, W], f32)
                nc.vector.tensor_copy(t_nat[:], t_i32[:])
                t_natA = pool.tile([P, 2, W], f32)
                nc.scalar.copy(t_natA[:], t_nat[:])

                t_T = pool.tile([P, 2, H], sdt)
                for wb in range(2):
                    pt = psum.tile([P, H], f32, tag="pt")
                    for hb in range(2):
                        nc.tensor.transpose(
                            pt[:, hb * P:(hb + 1) * P],
                            t_natA[:, hb, wb * P:(wb + 1) * P], ident[:])
                    nc.scalar.copy(t_T[:, wb, :], pt[:])

                # ---- softmax pieces (bf16): e = exp(pred), q = 1/sum ----
                pred = dpool.tile([P, 2, C, W], f32)
                pred_v = pred_d[b].rearrange("c (n p) w -> p n c w", p=P)
                for hb in range(2):
                    nc.sync.dma_start(pred[:, hb], pred_v[:, hb])
                e_all = pool.tile([P, 2, C, W], bf16)
                nc.scalar.activation(e_all[:], pred[:], Act.Exp)

                # ---- per-class not-present masks + scans along h ----
                notpA = pool.tile([P, C, 2, SCW], sdt)
                for c in range(C):
                    nc.vector.tensor_scalar(
                        notpA[:, c, :, :H], t_T[:], float(c), None,
                        Alu.not_equal)
                nc.vector.memset(notpA[:, :, :, H:], 1.0e6)
                if b == 0:
                    nc.vector.memset(ones_s[:], 1.0)
                fwdA = pool.tile([P, C, 2, SCW], sdt)
                bwdA = pool.tile([P, C, 2, SCW], sdt)
                nfl = notpA[:].rearrange("p c a h -> p (c a h)")
                nc.vector.tensor_tensor_scan(
                    fwdA[:].rearrange("p c a h -> p (c a h)"),
                    ones_s[:], nfl, BIG, Alu.add, Alu.mult)
                nc.vector.tensor_tensor_scan(
                    bwdA[:].rearrange("p c a h -> p (c a h)")[:, ::-1],
                    ones_s[:], nfl[:, ::-1], BIG, Alu.add, Alu.mult)
                gmin = pool.tile([P, C, 2, H], bf16)
                nc.vector.tensor_tensor(
                    gmin[:], fwdA[:, :, :, :H], bwdA[:, :, :, :H], Alu.min)

                # softmax sums/normalization issued after the scan chain so
                # the DVE never idles waiting on ACT's exp
                sA = pool.tile([P, 2, W], bf16)
                sB = pool.tile([P, 2, W], bf16)
                nc.vector.tensor_tensor(
                    sA[:], e_all[:, :, 0, :], e_all[:, :, 1, :], Alu.add)
                nc.vector.tensor_tensor(
                    sB[:], e_all[:, :, 2, :], e_all[:, :, 3, :], Alu.add)
                nc.vector.tensor_tensor(sA[:], sA[:], sB[:], Alu.add)
                nc.vector.tensor_tensor(
                    sA[:], sA[:], e_all[:, :, 4, :], Alu.add)
                lg = pool.tile([P, 2, W], f32)
                nc.scalar.activation(lg[:], sA[:], Act.Ln)
                q = pool.tile([P, 2, W], bf16)
                nc.scalar.activation(q[:], lg[:], Act.Exp, scale=-1.0)
                eq = pool.tile([P, 2, C, W], bf16)
                nc.vector.tensor_tensor(
                    eq[:], e_all[:],
                    q[:].unsqueeze(2).broadcast_to([P, 2, C, W]), Alu.mult)

                # ---- transpose each class map back; Square fused into the
                #      PSUM->SBUF ACT copy. Flat layout: 10 chunks of
                #      [8 pads | 256 data] + 16 trailing pads, so every
                #      shifted read lands in 1e9 pads and every TT tap is
                #      4B-aligned (even element offsets only). ----
                F = TL + 16  # 2656
                G2b = pool.tile([P, F], bf16)
                # leading pads of each chunk + trailing pads
                nc.vector.memset(
                    G2b[:, :TL].rearrange("p (k w) -> p k w", w=CW)[:, :, :8],
                    1.0e9)
                nc.vector.memset(G2b[:, TL:], 1.0e9)
                gminA = pool.tile([P, C, 2, H], bf16)
                nc.scalar.copy(gminA[:], gmin[:])
                for c in range(C):
                    for hb in range(2):
                        pg = psum.tile([P, W], bf16, tag="pg")
                        for wb in range(2):
                            nc.tensor.transpose(
                                pg[:, wb * P:(wb + 1) * P],
                                gminA[:, c, wb, hb * P:(hb + 1) * P], identB[:])
                        base = (hb * C + c) * CW + 8
                        nc.scalar.activation(
                            G2b[:, base:base + W], pg[:], Act.Square)

                # ---- banded min-plus along w, all taps as aligned 2x TT.
                # Even offsets read unshifted B_k = G2 + k^2 (TS, 4x-packed)
                # through even-shifted views; odd offsets read shifted
                # pre-biased copies Bo = G2[j-+k] + k^2 built on ACT (ACT has
                # no packing/alignment constraint), so no 1x STT remains. ----
                G2bf = G2b[:]
                d2p = pool.tile([P, F], bf16)
                d2pf = d2p[:]
                bk = pool.tile([P, F], bf16, tag="bias")
                uk = pool.tile([P, F], bf16, tag="upair")
                # delta = +-2 (B2 = G2 + 4); first tap doubles as the init
                nc.vector.tensor_scalar(bk[:], G2bf, 4.0, None, Alu.add)
                nc.vector.tensor_tensor(
                    d2pf[:, 2:], bk[:, :F - 2], G2bf[:, 2:], Alu.min)
                nc.vector.tensor_tensor(
                    d2pf[:, :F - 2], bk[:, 2:], d2pf[:, :F - 2], Alu.min)
                # delta = +-4, +-6
                for dlt in (4, 6):
                    nc.vector.tensor_scalar(
                        bk[:], G2bf, float(dlt * dlt), None, Alu.add)
                    nc.vector.tensor_tensor(
                        d2pf[:, dlt:], bk[:, :F - dlt], d2pf[:, dlt:], Alu.min)
                    nc.vector.tensor_tensor(
                        d2pf[:, :F - dlt], bk[:, dlt:], d2pf[:, :F - dlt],
                        Alu.min)
                # odd deltas 1,3,5: U_k[j] = min(G2[j], G2[j+2k]) (aligned TT),
                # then one STT tap d2p[j] = min(U_k[j-k] + k^2, d2p[j])
                for dlt in (1, 3, 5):
                    nc.vector.tensor_tensor(
                        uk[:, :F - 2 * dlt], G2bf[:, :F - 2 * dlt],
                        G2bf[:, 2 * dlt:], Alu.min)
                    nc.vector.scalar_tensor_tensor(
                        d2pf[:, dlt:], uk[:, :F - dlt], float(dlt * dlt),
                        d2pf[:, dlt:], Alu.add, Alu.min)

                # ---- per-slice max(d2): one strided X-reduce -> [P, 10] ----
                col = 20 + b * 10
                nc.vector.tensor_reduce(
                    stats[:, col:col + 10],
                    d2p[:, :TL].rearrange("p (k w) -> p k w", w=CW)[:, :, 8:],
                    mybir.AxisListType.X, Alu.max)

                # ---- partial sums ----
                # S1_c = sum(sqrt(d2)*eq) = sum(sqrt(d2*eq^2)): the per-class
                # product runs as a 2x TT on DVE, the sqrt+sum on ACT (accum).
                d2pV = d2p[:, :TL].rearrange("p (a c w) -> p a c w", a=2, c=C)
                eq2 = pool.tile([P, 2, C, W], bf16)
                nc.scalar.activation(eq2[:], eq[:], Act.Square)
                z0 = pool.tile([P, 2, W], bf16, tag="z0")
                z1 = pool.tile([P, 2, W], bf16, tag="z1")
                junks = pool.tile([P, 2, W], bf16, tag="junks")
                junkb = pool.tile([P, 2, W], bf16, tag="junkb")
                for c in range(C):
                    c1 = b * C + c
                    z = z0 if c % 2 == 0 else z1
                    nc.vector.tensor_tensor(
                        z[:], d2pV[:, :, c, 8:], eq2[:, :, c, :], Alu.mult)
                    nc.scalar.activation(
                        junks[:], z[:], Act.Sqrt,
                        accum_out=stats[:, c1:c1 + 1])
                    nc.vector.scalar_tensor_tensor(
                        junkb[:], d2pV[:, :, c, 8:], 0.5, eq[:, :, c, :],
                        Alu.is_lt, Alu.mult,
                        accum_out=stats[:, 10 + c1:11 + c1])

            nc.sync.dma_start(out_d[:], stats[:])

    # walrus single-wait + range-clear fixups
    rc_op = nc.isa.Opcode.NEURON_ISA_TPB_OPCODE_EVENT_SEMAPHORE_RANGE_CLEAR.value
    for f in nc.m.functions:
        for blk in f.blocks:
            newlist = []
            for inst in blk.instructions:
                si = inst.sync_info
                if si is not None and si.on_wait and len(si.on_wait) > 1:
                    for w in si.on_wait[:-1]:
                        newlist.append(mybir.InstNoOp(
                            name=nc.get_next_instruction_name(),
                            engine=inst.engine,
                            bass_nofuse=True,
                            sync_info=mybir.SyncInfo(on_wait=[w], on_update=[]),
                        ))
                    si.on_wait = [si.on_wait[-1]]
                if (isinstance(inst, mybir.InstISA)
                        and inst.isa_opcode == rc_op):
                    struct = inst.ant_dict
                    for semid in range(struct["range_first"],
                                       struct["range_last"] + 1):
                        newlist.append(mybir.InstNoOp(
                            name=nc.get_next_instruction_name(),
                            engine=inst.engine,
                            bass_nofuse=True,
                            sync_info=mybir.SyncInfo(
                                on_wait=list(si.on_wait) if (
                                    si and semid == struct["range_first"]
                                ) else [],
                                on_update=[mybir.SyncUpdate(
                                    sync_type="semaphore", id=semid,
                                    update_mode="sem-wr-imm",
                                    update_value=0)],
                            ),
                        ))
                    continue
                newlist.append(inst)
            blk.instructions[:] = newlist
    return nc


def _numpy_fallback(predictions, weight, targets):
    predictions = np.asarray(predictions, np.float32)
    targets = np.asarray(targets)
    weight = np.asarray(weight, np.float32)
    Bf, Cf, Hf, Wf = predictions.shape
    big = np.float32(Hf + Wf)
    total = np.float64(0.0)
    wn = (weight / weight.sum()).astype(np.float32)
    for b in range(Bf):
        pm = predictions[b] - predictions[b].max(axis=0, keepdims=True)
        ex = np.exp(pm, dtype=np.float32)
        probs = ex / ex.sum(axis=0, keepdims=True)
        for c in range(Cf):
            p = (targets[b] == c)
            notp = ~p
            fwd = np.zeros((Hf, Wf), np.float32)
            st = np.full((Hf,), big, np.float32)
            for t in range(Wf):
                st = np.where(notp[:, t], st + 1.0, 0.0)
                fwd[:, t] = st
            bwd = np.zeros((Hf, Wf), np.float32)
            st = np.full((Hf,), big, np.float32)
            for t in range(Wf - 1, -1, -1):
                st = np.where(notp[:, t], st + 1.0, 0.0)
                bwd[:, t] = st
            g = np.minimum(np.minimum(fwd, bwd), big)
            i = np.arange(Hf, dtype=np.float32)
            A = (i[:, None] - i[None, :]) ** 2
            d2 = (A[:, :, None] + (g * g)[None, :, :]).min(axis=1)
            d = np.sqrt(d2)
            dist = np.where(p, np.float32(-1.0) * d.max(), d)
            total += np.float64((probs[c] * dist).sum(dtype=np.float64)) * wn[c]
    return np.float32(total / (Bf * Cf * Hf * Wf))


def kernel(predictions, weight, targets):
    predictions = np.ascontiguousarray(np.asarray(predictions, np.float32))
    targets = np.ascontiguousarray(np.asarray(targets, np.int32))
    weight = np.asarray(weight, np.float32)

    safe_inputs = (
        np.all(np.isfinite(weight)) and np.all(weight > 0)
        and np.all(np.isfinite(predictions))
        and float(np.abs(predictions).max()) < 80.0
    )
    if not safe_inputs:
        return _numpy_fallback(predictions, weight, targets)

    from concourse.bass_utils import run_bass_kernel_spmd

    if "nc" not in _CACHE:
        _CACHE["nc"] = _build_nc()
    nc = _CACHE["nc"]

    in_maps = [
        {
            "predictions": predictions[i * BPC:(i + 1) * BPC],
            "targets": targets[i * BPC:(i + 1) * BPC],
        }
        for i in range(NCORES)
    ]
    res = run_bass_kernel_spmd(nc, in_maps, core_ids=list(range(NCORES)))
    stats = np.stack([r["out_stats"] for r in res.results])  # [8, 128, 48]

    S1 = stats[:, :, 0:10].sum(axis=1, dtype=np.float64).reshape(NCORES, BPC, C)
    S2 = stats[:, :, 10:20].sum(axis=1, dtype=np.float64).reshape(NCORES, BPC, C)
    # maxd2 cols: 20 + b*10 + a*5 + c
    m = stats[:, :, 20:40].reshape(NCORES, P, BPC, 2, C)
    maxd2 = m.max(axis=(1, 3))  # [8, BPC, C]

    if maxd2.max() > float(R * R):
        return _numpy_fallback(predictions, weight, targets)

    M = np.sqrt(maxd2.astype(np.float32)).astype(np.float64)
    wn = (weight / weight.sum()).astype(np.float64)
    loss = ((S1 - M * S2) * wn[None, None, :]).sum() / float(B * C * H * W)
    return np.float32(loss)
